# revision 11
# baseline (speedup 1.0000x reference)
"""GAT layer (PyG GATConv semantics) on 8 Trainium2 NeuronCores.

Strategy (edge/graph parallel, dst-sharded):
  - Append self-loops; partition destination nodes into 784 windows of 128.
  - Rank windows by edge count; window rank-group g supplies slot g of each
    of the 8 cores, so all cores share one compile-time schedule.
  - Each core: builds the full node table T1[n] = [h(n) | a_src(n)] (bf16,
    512B rows) from x @ [W | W@A] on the TensorEngine, plus a local
    per-dst-shard table T2 = [a_dst | pad] (bf16, 256B rows).
  - Edge phase: dma_gather rows of T1 by edge src (4 table chunks, int16
    local indices) and rows of T2 by edge dst; per 128-edge tile build a
    one-hot selection matrix Sel[e,d] = (dstloc_e == d) and accumulate
      psum[d, 0:128] += Sel.T @ (w ⊙ h_src);  psum[d, 128:132] += Sel.T @ w
    where w = exp(leaky_relu(a_src[src] + a_dst[dst])) = max(exp(z),
    exp(0.2 z)).  Finally out[d] = psum[d]/s[d] + bias, written per window.
"""

import math
import os

import numpy as np
import ml_dtypes

import concourse.bacc as bacc
import concourse.bass as bass
import concourse.mybir as mybir
import concourse.tile as tile
from concourse.library_config import mlp
from concourse.bass_utils import run_bass_kernel_spmd
from concourse.masks import make_identity
from concourse.vector_clock import ScopedClock

BF16 = ml_dtypes.bfloat16

N = 100000
E = 1600000
IN_DIM = 128
HEADS = 4
CDIM = 32
NCORES = 8
P = 128

NP_ = 100352            # N padded to 784 x-tiles of 128
NWIN = NP_ // P         # 784 global windows
SLOTS = NWIN // NCORES  # 98 slots per core
CHUNK = NP_ // 4        # 25088 rows per T1 chunk (int16-indexable)
SHARD = SLOTS * P       # 12544 dst nodes per core
PADROW = SHARD          # T2 pad row (a_dst = -60 => w ~ 0)
SUPB = 8                # slots per superblock (gather batching)

_NEG = -60.0


# ---------------------------------------------------------------------------
# walrus workaround: this container's walrus accepts ONE sem wait per
# instruction; TileContext's tail drain accumulates many. Split extras onto
# single-wait EventSemaphore instructions.
def _split_multi_waits(nc):
    n = [0]

    def fresh():
        n[0] += 1
        return f"waitsplit-{n[0]}"

    for fn in nc.m.functions:
        for bb in fn.blocks:
            insts = list(bb.instructions)
            if not any(
                i.sync_info is not None and len(i.sync_info.on_wait) > 1
                for i in insts
            ):
                continue
            out = []
            for inst in insts:
                si = inst.sync_info
                if si is not None and len(si.on_wait) > 1:
                    waits = list(si.on_wait)
                    for w in waits[:-1]:
                        out.append(mybir.InstEventSemaphore(
                            name=fresh(), opcode="EventSemaphore",
                            engine=inst.engine,
                            sync_info=mybir.SyncInfo(on_wait=[w], on_update=[]),
                        ))
                    si.on_wait = waits[-1:]
                out.append(inst)
            bb.instructions = out


def _wrap_idx(seg):
    """dma_gather index layout: wrap in 16 partitions, replicate x8."""
    assert seg.size % 128 == 0
    return np.tile(seg.reshape(-1, 16).T, (8, 1)).astype(np.int16)


# ---------------------------------------------------------------------------
def _host_prep(x, edge_index):
    """Build the per-core schedule + data arrays. Pure indexing, no FP math."""
    src = np.concatenate([edge_index[0].astype(np.int64), np.arange(N)])
    dst = np.concatenate([edge_index[1].astype(np.int64), np.arange(N)])
    win = dst >> 7

    wcount = np.bincount(win, minlength=NWIN)
    order = np.argsort(-wcount, kind="stable")        # windows by size desc
    core_of_win = np.empty(NWIN, np.int64)
    slot_of_win = np.empty(NWIN, np.int64)
    core_of_win[order] = np.arange(NWIN) % NCORES
    slot_of_win[order] = np.arange(NWIN) // NCORES

    chunk = src // CHUNK
    wc = np.bincount(win * 4 + chunk, minlength=NWIN * 4).reshape(NWIN, 4)
    # caps[g][c]: tiles for chunk-c segment of slot g (max over the 8 cores)
    grp = order.reshape(SLOTS, NCORES)
    caps = np.ceil(wc[grp].max(axis=1) / P).astype(np.int64)   # [SLOTS, 4]
    caps = np.maximum(caps, 0)

    # stream layout: position ordered by (supb, chunk, slot, tile, lane)
    supb_sizes = [SUPB] * (SLOTS // SUPB) + ([SLOTS % SUPB] if SLOTS % SUPB else [])
    seg_tiles = []          # (s, c) -> tiles
    slot_seg_off = np.zeros((SLOTS, 4), np.int64)   # tile offset of (g, c) run
    tcursor = 0
    sb0 = 0
    for sb, nsl in enumerate(supb_sizes):
        for c in range(4):
            for j in range(nsl):
                g = sb0 + j
                slot_seg_off[g, c] = tcursor
                tcursor += caps[g, c]
            seg_tiles.append((sb, c, int(caps[sb0:sb0 + nsl, c].sum())))
        sb0 += nsl
    T_tot = tcursor

    # per-core arrays
    ecore = core_of_win[win]
    eslot = slot_of_win[win]
    cores = []
    for k in range(NCORES):
        m = np.nonzero(ecore == k)[0]
        es, ed, ec, eg = src[m], dst[m], chunk[m], eslot[m]
        o = np.lexsort((ed, ec, eg))
        es, ed, ec, eg = es[o], ed[o], ec[o], eg[o]
        # rank within (slot, chunk) group
        key = eg * 4 + ec
        start = np.searchsorted(key, np.arange(SLOTS * 4))
        rank = np.arange(len(es)) - start[key]
        pos = slot_seg_off[eg, ec] * P + rank
        g1 = np.zeros(T_tot * P, np.int16)                      # pad: row 0
        g2 = np.full(T_tot * P, PADROW, np.int16)               # pad: -60 row
        dl = np.zeros(T_tot * P, np.int16)                      # pad: d 0
        g1[pos] = (es - ec * CHUNK).astype(np.int16)
        g2[pos] = (eg * P + (ed & 127)).astype(np.int16)
        dl[pos] = (ed & 127).astype(np.int16)
        cores.append({"g1": g1, "g2": g2, "dl": dl})

    sched = {
        "caps": caps, "supb_sizes": supb_sizes, "seg_tiles": seg_tiles,
        "T_tot": T_tot, "order": order, "grp": grp,
        "core_of_win": core_of_win, "slot_of_win": slot_of_win,
    }
    return cores, sched


def _pack_core_arrays(core, sched):
    """Wrap index streams per gather instruction; dstloc per tile column."""
    T_tot = sched["T_tot"]
    g1_parts, g2_parts = [], []
    t0 = 0
    for (sb, c, tiles) in sched["seg_tiles"]:
        seg = core["g1"][t0 * P:(t0 + tiles) * P]
        if tiles:
            g1_parts.append(_wrap_idx(seg))
        t0 += tiles
    # g2: same per-(supb, chunk) segmentation as g1 (descriptor-ring cap)
    t0 = 0
    for (sb, c, tiles) in sched["seg_tiles"]:
        seg = core["g2"][t0 * P:(t0 + tiles) * P]
        if tiles:
            g2_parts.append(_wrap_idx(seg))
        t0 += tiles
    g1w = np.concatenate(g1_parts, axis=1) if g1_parts else np.zeros((128, 0), np.int16)
    g2w = np.concatenate(g2_parts, axis=1) if g2_parts else np.zeros((128, 0), np.int16)
    dl = core["dl"].reshape(T_tot, P).T.astype(BF16).copy()
    return g1w, g2w, dl


# ---------------------------------------------------------------------------
def _build_nc(sched):
    caps = sched["caps"]
    supb_sizes = sched["supb_sizes"]
    T_tot = sched["T_tot"]
    AF = mybir.ActivationFunctionType
    AL = mybir.AluOpType
    f32, bf16, i16 = mybir.dt.float32, mybir.dt.bfloat16, mybir.dt.int16

    g1cols = sum(t * 8 for (_, _, t) in sched["seg_tiles"])
    g2cols = T_tot * 8

    nc = bacc.Bacc("TRN2")
    xT = nc.declare_dram_parameter("xT", [P, NP_], f32, isOutput=False)
    xsT = nc.declare_dram_parameter("xsT", [P, SHARD], f32, isOutput=False)
    Wp = nc.declare_dram_parameter("W", [P, P], f32, isOutput=False)
    Acat = nc.declare_dram_parameter("Acat", [P, 8], f32, isOutput=False)
    biasr = nc.declare_dram_parameter("biasr", [P, P], f32, isOutput=False)
    iotap = nc.declare_dram_parameter("iota", [P, P], bf16, isOutput=False)
    negrow = nc.declare_dram_parameter("negrow", [1, P], bf16, isOutput=False)
    g1i = nc.declare_dram_parameter("g1i", [P, max(g1cols, 8)], i16, isOutput=False)
    g2i = nc.declare_dram_parameter("g2i", [P, max(g2cols, 8)], i16, isOutput=False)
    dlp = nc.declare_dram_parameter("dloc", [P, max(T_tot, 1)], bf16, isOutput=False)
    outp = nc.declare_dram_parameter("out", [SHARD, P], f32, isOutput=True)

    T1 = nc.dram_tensor("t1", [NP_, 256], bf16)
    T2 = nc.dram_tensor("t2", [SHARD + 1, P], bf16)

    nc.gpsimd.load_library(mlp)

    with tile.TileContext(nc) as tc:
        with tc.tile_pool(name="const", bufs=1) as cpool:
            ident = cpool.tile([P, P], f32)
            make_identity(nc, ident[:])
            iot = cpool.tile([P, P], bf16)
            nc.sync.dma_start(out=iot[:], in_=iotap[:])
            bias_t = cpool.tile([P, P], f32)
            nc.sync.dma_start(out=bias_t[:], in_=biasr[:])
            wprime = cpool.tile([P, 136], f32)
            nc.sync.dma_start(out=wprime[:, 0:128], in_=Wp[:])
            acat_t = cpool.tile([P, 8], f32)
            nc.sync.dma_start(out=acat_t[:], in_=Acat[:])
            # ---------------- table build ----------------
            with tc.tile_pool(name="tb", bufs=3) as tb, \
                 tc.tile_pool(name="tbp", bufs=2, space="PSUM") as tbp:
                # W' cols 128:136 = W @ Acat  (contract over out-features)
                wtp = tbp.tile([P, P], f32, space="PSUM")
                nc.tensor.transpose(out=wtp[:], in_=wprime[:, 0:128], identity=ident[:])
                wT = tb.tile([P, P], f32)
                nc.vector.tensor_copy(out=wT[:], in_=wtp[:])
                wap = tbp.tile([P, 8], f32, space="PSUM")
                nc.tensor.matmul(out=wap[:], lhsT=wT[:], rhs=acat_t[:],
                                 start=True, stop=True)
                nc.vector.tensor_copy(out=wprime[:, 128:136], in_=wap[:])
                NBLK = 8
                for b in range(NP_ // P // NBLK):
                    xt = tb.tile([P, NBLK * P], f32, tag="xt")
                    nc.sync.dma_start(
                        out=xt[:], in_=xT[:, b * NBLK * P:(b + 1) * NBLK * P])
                    st = tb.tile([P, NBLK * 256], bf16, tag="st")
                    nc.gpsimd.memset(st[:], 0)
                    for t in range(NBLK):
                        ps = tbp.tile([P, 136], f32, space="PSUM", tag="ps")
                        nc.tensor.matmul(
                            out=ps[:], lhsT=xt[:, t * P:(t + 1) * P],
                            rhs=wprime[:], start=True, stop=True)
                        if t % 2 == 0:
                            nc.vector.tensor_copy(
                                out=st[:, t * 256:t * 256 + 136], in_=ps[:])
                        else:
                            nc.scalar.activation(
                                out=st[:, t * 256:t * 256 + 136], in_=ps[:],
                                func=AF.Copy)
                    nc.sync.dma_start(
                        out=T1[b * NBLK * P:(b + 1) * NBLK * P, :].rearrange(
                            "(t p) c -> p t c", p=P),
                        in_=st[:].rearrange("p (t c) -> p t c", t=NBLK))
                for j in range(SLOTS):
                    xt = tb.tile([P, P], f32, tag="xt")
                    nc.sync.dma_start(out=xt[:], in_=xsT[:, j * P:(j + 1) * P])
                    ps = tbp.tile([P, 136], f32, space="PSUM", tag="ps")
                    nc.tensor.matmul(out=ps[:], lhsT=xt[:], rhs=wprime[:],
                                     start=True, stop=True)
                    st2 = tb.tile([P, P], bf16, tag="st2")
                    nc.gpsimd.memset(st2[:, 4:128], 0)
                    nc.vector.tensor_copy(out=st2[:, 0:4], in_=ps[:, 132:136])
                    nc.sync.dma_start(out=T2[j * P:(j + 1) * P, :], in_=st2[:])
                ng = tb.tile([1, P], bf16, tag="ng")
                nc.sync.dma_start(out=ng[:], in_=negrow[:])
                nc.sync.dma_start(out=T2[PADROW:PADROW + 1, :], in_=ng[:])

            # ---------------- edge phase ----------------
            _PH = int(os.environ.get("GAT_PHASES", "3"))
            with tc.tile_pool(name="eg", bufs=2) as eg, \
                 tc.tile_pool(name="ew", bufs=4) as ew, \
                 tc.tile_pool(name="eo", bufs=2) as eo, \
                 tc.tile_pool(name="eps", bufs=1, space="PSUM") as epsum:
                g1_colcur = 0
                g2_colcur = 0
                tilecur = 0
                seg_iter = 0
                sb0 = 0
                for sb, nsl in enumerate(supb_sizes):
                    if _PH < 1:
                        break
                    sl = slice(sb0, sb0 + nsl)
                    ctiles = [int(caps[sl, c].sum()) for c in range(4)]
                    stiles = sum(ctiles)
                    if stiles == 0:
                        sb0 += nsl
                        seg_iter += 4
                        continue
                    # index + dstloc loads for this superblock
                    g2it = eg.tile([P, stiles * 8], i16, tag="g2it")
                    nc.scalar.dma_start(
                        out=g2it[:], in_=g2i[:, g2_colcur:g2_colcur + stiles * 8])
                    g2_colcur += stiles * 8
                    dlt = eg.tile([P, stiles], bf16, tag="dlt")
                    nc.scalar.dma_start(
                        out=dlt[:], in_=dlp[:, tilecur:tilecur + stiles])
                    # a_dst gather buffer; filled per chunk segment below
                    g2b = eg.tile([P, stiles * P], bf16, tag="g2b")
                    if _PH < 2:
                        nc.gpsimd.memset(g2b[:], 0)

                    psum_of_slot = {}
                    flags = {}
                    for j in range(nsl):
                        g = sb0 + j
                        live = [c for c in range(4) if caps[g, c] > 0]
                        if live:
                            flags[j] = (live[0], live[-1])
                            psum_of_slot[j] = epsum.tile(
                                [P, 132], f32, space="PSUM", name=f"pslot{j}", tag=f"ps{j}")

                    srun = 0   # tile index within the supb stream
                    for c in range(4):
                        Lc = ctiles[c]
                        if Lc == 0:
                            seg_iter += 1
                            continue
                        g1it = eg.tile([P, Lc * 8], i16, tag="g1it")
                        nc.scalar.dma_start(
                            out=g1it[:], in_=g1i[:, g1_colcur:g1_colcur + Lc * 8])
                        g1_colcur += Lc * 8
                        g1b = eg.tile([P, Lc * 256], bf16, tag="g1b", bufs=3)
                        if _PH < 2:
                            nc.gpsimd.memset(g1b[:], 0)
                        _PH < 2 or nc.gpsimd.dma_gather(
                            g1b[:].rearrange("p (t c) -> p t c", t=Lc),
                            T1[c * CHUNK:(c + 1) * CHUNK, :],
                            g1it[:], Lc * P, Lc * P, 256,
                            single_packet=False)
                        _PH < 2 or nc.gpsimd.dma_gather(
                            g2b[:].rearrange("p (t c) -> p t c", t=stiles)[
                                :, srun:srun + Lc, :],
                            T2[:], g2it[:, srun * 8:(srun + Lc) * 8],
                            Lc * P, Lc * P, P, single_packet=False)
                        g1v = g1b[:].rearrange("p (t c) -> p t c", t=Lc)
                        crun = 0   # tile within this chunk segment
                        for j in range(nsl):
                            g = sb0 + j
                            K = int(caps[g, c])
                            if K == 0:
                                continue
                            po = psum_of_slot[j]
                            # logits/weights for the whole run
                            z = ew.tile([P, K * 4], f32, tag="z")
                            nc.vector.tensor_tensor(
                                out=z[:].rearrange("p (t c) -> p t c", t=K),
                                in0=g1v[:, crun:crun + K, 128:132],
                                in1=g2b[:].rearrange("p (t c) -> p t c", t=stiles)[
                                    :, srun + crun:srun + crun + K, 0:4],
                                op=AL.add)
                            e1 = ew.tile([P, K * 4], f32, tag="e1")
                            nc.scalar.activation(out=e1[:], in_=z[:], func=AF.Exp)
                            e2 = ew.tile([P, K * 4], f32, tag="e2")
                            nc.scalar.activation(out=e2[:], in_=z[:], func=AF.Exp,
                                                 scale=0.2)
                            w = ew.tile([P, K * 4], f32, tag="w")
                            nc.vector.tensor_tensor(out=w[:], in0=e1[:], in1=e2[:],
                                                    op=AL.max)
                            wv = w[:].rearrange("p (t c) -> p t c", t=K)
                            for t in range(K):
                                gt = crun + t
                                sel = ew.tile([P, P], bf16, tag="sel")
                                nc.vector.tensor_tensor(
                                    out=sel[:],
                                    in0=dlt[:, srun + gt:srun + gt + 1].to_broadcast([P, P]),
                                    in1=iot[:], op=AL.is_equal)
                                mp = ew.tile([P, 132], bf16, tag="mp")
                                nc.vector.tensor_tensor(
                                    out=mp[:, 0:128].rearrange("p (h c) -> p h c", h=4),
                                    in0=g1v[:, gt, 0:128].rearrange("p (h c) -> p h c", h=4),
                                    in1=wv[:, t, :].unsqueeze(-1).to_broadcast([P, 4, 32]),
                                    op=AL.mult)
                                nc.vector.tensor_copy(out=mp[:, 128:132], in_=wv[:, t, :])
                                fc = flags[j]
                                nc.tensor.matmul(
                                    out=po[:], lhsT=sel[:], rhs=mp[:],
                                    start=(c == fc[0] and t == 0),
                                    stop=(c == fc[1] and t == K - 1))
                            crun += K
                        srun += Lc
                        seg_iter += 1
                    # flush slots
                    for j in range(nsl):
                        if j not in psum_of_slot:
                            continue
                        g = sb0 + j
                        po = psum_of_slot[j]
                        rec = ew.tile([P, 4], f32, tag="rec")
                        nc.vector.reciprocal(out=rec[:], in_=po[:, 128:132])
                        ot = eo.tile([P, P], f32, tag="ot")
                        for hh in range(4):
                            nc.vector.tensor_scalar_mul(
                                ot[:, hh * 32:(hh + 1) * 32],
                                po[:, hh * 32:(hh + 1) * 32], rec[:, hh:hh + 1])
                        nc.vector.tensor_tensor(out=ot[:], in0=ot[:], in1=bias_t[:],
                                                op=AL.add)
                        nc.sync.dma_start(out=outp[g * P:(g + 1) * P, :], in_=ot[:])
                    tilecur += stiles
                    sb0 += nsl
    nc.compile()
    if not os.environ.get("BASS_NO_WAITSPLIT"):
        _split_multi_waits(nc)
    return nc


# ---------------------------------------------------------------------------
_BUILD_CACHE = {}


def _prep_and_build(x, edge_index, W, att_src, att_dst, bias):
    cores, sched = _host_prep(np.asarray(x), np.asarray(edge_index))
    nc = _build_nc(sched)

    x = np.asarray(x, np.float32)
    xpad = np.zeros((NP_, IN_DIM), np.float32)
    xpad[:N] = x
    xT = np.ascontiguousarray(xpad.T)

    Acat = np.zeros((P, 8), np.float32)
    a_s = np.asarray(att_src, np.float32)
    a_d = np.asarray(att_dst, np.float32)
    for h in range(HEADS):
        Acat[h * CDIM:(h + 1) * CDIM, h] = a_s[h]
        Acat[h * CDIM:(h + 1) * CDIM, 4 + h] = a_d[h]
    biasr = np.tile(np.asarray(bias, np.float32)[None, :], (P, 1))
    iota = np.tile(np.arange(P, dtype=BF16)[None, :], (P, 1))
    negrow = np.full((1, P), _NEG, BF16)
    Wf = np.ascontiguousarray(np.asarray(W, np.float32))

    in_maps = []
    for k in range(NCORES):
        g1w, g2w, dl = _pack_core_arrays(cores[k], sched)
        nodes = (sched["grp"][:, k][:, None] * P + np.arange(P)[None, :]).reshape(-1)
        xsT = np.ascontiguousarray(xpad[nodes].T)
        in_maps.append({
            "xT": xT, "xsT": xsT, "W": Wf, "Acat": Acat, "biasr": biasr,
            "iota": iota, "negrow": negrow,
            "g1i": np.ascontiguousarray(g1w), "g2i": np.ascontiguousarray(g2w),
            "dloc": np.ascontiguousarray(dl),
        })
    return nc, in_maps, sched


def _assemble(results, sched):
    full = np.zeros((NP_, P), np.float32)
    grp = sched["grp"]
    for k in range(NCORES):
        o = np.asarray(results[k]["out"])        # [SHARD, 128]
        wins = grp[:, k]                         # window id per slot
        full[(wins[:, None] * P + np.arange(P)[None, :]).reshape(-1)] = o
    return full[:N]


def kernel(**inputs):
    x = inputs["x"]
    edge_index = inputs["edge_index"]
    nc, in_maps, sched = _prep_and_build(
        x, edge_index, inputs["W"], inputs["att_src"], inputs["att_dst"],
        inputs["bias"])
    res = run_bass_kernel_spmd(nc, in_maps, core_ids=list(range(NCORES)))
    return _assemble(res.results, sched)


# revision 12
# speedup vs baseline: 5.4103x; 5.4103x over previous
"""GAT layer (PyG GATConv semantics) on 8 Trainium2 NeuronCores.

Strategy (edge/graph parallel, dst-sharded):
  - Append self-loops; partition destination nodes into 784 windows of 128.
  - Rank windows by edge count; window rank-group g supplies slot g of each
    of the 8 cores, so all cores share one compile-time schedule.
  - Each core: builds the full node table T1[n] = [h(n) | a_src(n)] (bf16,
    512B rows) from x @ [W | W@A] on the TensorEngine, plus a local
    per-dst-shard table T2 = [a_dst | pad] (bf16, 256B rows).
  - Edge phase: dma_gather rows of T1 by edge src (4 table chunks, int16
    local indices) and rows of T2 by edge dst; per 128-edge tile build a
    one-hot selection matrix Sel[e,d] = (dstloc_e == d) and accumulate
      psum[d, 0:128] += Sel.T @ (w ⊙ h_src);  psum[d, 128:132] += Sel.T @ w
    where w = exp(leaky_relu(a_src[src] + a_dst[dst])) = max(exp(z),
    exp(0.2 z)).  Finally out[d] = psum[d]/s[d] + bias, written per window.
"""

import math
import os

import numpy as np
import ml_dtypes

import concourse.bacc as bacc
import concourse.bass as bass
import concourse.mybir as mybir
import concourse.tile as tile
from concourse.library_config import mlp
from concourse.bass_utils import run_bass_kernel_spmd
from concourse.masks import make_identity
from concourse.vector_clock import ScopedClock

BF16 = ml_dtypes.bfloat16

N = 100000
E = 1600000
IN_DIM = 128
HEADS = 4
CDIM = 32
NCORES = 8
P = 128

NP_ = 100352            # N padded to 784 x-tiles of 128
NWIN = NP_ // P         # 784 global windows
SLOTS = NWIN // NCORES  # 98 slots per core
CHUNK = NP_ // 4        # 25088 rows per T1 chunk (int16-indexable)
SHARD = SLOTS * P       # 12544 dst nodes per core
PADROW = SHARD          # T2 pad row (a_dst = -60 => w ~ 0)
SUPB = 8                # slots per superblock (gather batching)

_NEG = -60.0


# ---------------------------------------------------------------------------
# walrus workaround: this container's walrus accepts ONE sem wait per
# instruction; TileContext's tail drain accumulates many. Split extras onto
# single-wait EventSemaphore instructions.
def _split_multi_waits(nc):
    n = [0]

    def fresh():
        n[0] += 1
        return f"waitsplit-{n[0]}"

    for fn in nc.m.functions:
        for bb in fn.blocks:
            insts = list(bb.instructions)
            if not any(
                i.sync_info is not None and len(i.sync_info.on_wait) > 1
                for i in insts
            ):
                continue
            out = []
            for inst in insts:
                si = inst.sync_info
                if si is not None and len(si.on_wait) > 1:
                    waits = list(si.on_wait)
                    for w in waits[:-1]:
                        out.append(mybir.InstEventSemaphore(
                            name=fresh(), opcode="EventSemaphore",
                            engine=inst.engine,
                            sync_info=mybir.SyncInfo(on_wait=[w], on_update=[]),
                        ))
                    si.on_wait = waits[-1:]
                out.append(inst)
            bb.instructions = out


def _wrap_idx(seg):
    """dma_gather index layout: wrap in 16 partitions, replicate x8."""
    assert seg.size % 128 == 0
    return np.tile(seg.reshape(-1, 16).T, (8, 1)).astype(np.int16)


# ---------------------------------------------------------------------------
def _host_prep(x, edge_index):
    """Build the per-core schedule + data arrays. Pure indexing, no FP math."""
    src = np.concatenate([edge_index[0].astype(np.int64), np.arange(N)])
    dst = np.concatenate([edge_index[1].astype(np.int64), np.arange(N)])
    win = dst >> 7

    wcount = np.bincount(win, minlength=NWIN)
    order = np.argsort(-wcount, kind="stable")        # windows by size desc
    core_of_win = np.empty(NWIN, np.int64)
    slot_of_win = np.empty(NWIN, np.int64)
    core_of_win[order] = np.arange(NWIN) % NCORES
    slot_of_win[order] = np.arange(NWIN) // NCORES

    chunk = src // CHUNK
    wc = np.bincount(win * 4 + chunk, minlength=NWIN * 4).reshape(NWIN, 4)
    # caps[g][c]: tiles for chunk-c segment of slot g (max over the 8 cores)
    grp = order.reshape(SLOTS, NCORES)
    caps = np.ceil(wc[grp].max(axis=1) / P).astype(np.int64)   # [SLOTS, 4]
    caps = np.maximum(caps, 0)

    # stream layout: position ordered by (supb, chunk, slot, tile, lane)
    supb_sizes = [SUPB] * (SLOTS // SUPB) + ([SLOTS % SUPB] if SLOTS % SUPB else [])
    seg_tiles = []          # (s, c) -> tiles
    slot_seg_off = np.zeros((SLOTS, 4), np.int64)   # tile offset of (g, c) run
    tcursor = 0
    sb0 = 0
    for sb, nsl in enumerate(supb_sizes):
        for c in range(4):
            for j in range(nsl):
                g = sb0 + j
                slot_seg_off[g, c] = tcursor
                tcursor += caps[g, c]
            seg_tiles.append((sb, c, int(caps[sb0:sb0 + nsl, c].sum())))
        sb0 += nsl
    T_tot = tcursor

    # per-core arrays
    ecore = core_of_win[win]
    eslot = slot_of_win[win]
    cores = []
    for k in range(NCORES):
        m = np.nonzero(ecore == k)[0]
        es, ed, ec, eg = src[m], dst[m], chunk[m], eslot[m]
        o = np.lexsort((ed, ec, eg))
        es, ed, ec, eg = es[o], ed[o], ec[o], eg[o]
        # rank within (slot, chunk) group
        key = eg * 4 + ec
        start = np.searchsorted(key, np.arange(SLOTS * 4))
        rank = np.arange(len(es)) - start[key]
        pos = slot_seg_off[eg, ec] * P + rank
        g1 = np.zeros(T_tot * P, np.int16)                      # pad: row 0
        g2 = np.full(T_tot * P, PADROW, np.int16)               # pad: -60 row
        dl = np.zeros(T_tot * P, np.int16)                      # pad: d 0
        g1[pos] = (es - ec * CHUNK).astype(np.int16)
        g2[pos] = (eg * P + (ed & 127)).astype(np.int16)
        dl[pos] = (ed & 127).astype(np.int16)
        cores.append({"g1": g1, "g2": g2, "dl": dl})

    sched = {
        "caps": caps, "supb_sizes": supb_sizes, "seg_tiles": seg_tiles,
        "T_tot": T_tot, "order": order, "grp": grp,
        "core_of_win": core_of_win, "slot_of_win": slot_of_win,
    }
    return cores, sched


def _pack_core_arrays(core, sched):
    """Wrap index streams per gather instruction; dstloc per tile column."""
    T_tot = sched["T_tot"]
    g1_parts, g2_parts = [], []
    t0 = 0
    for (sb, c, tiles) in sched["seg_tiles"]:
        seg = core["g1"][t0 * P:(t0 + tiles) * P]
        if tiles:
            g1_parts.append(_wrap_idx(seg))
        t0 += tiles
    # g2: same per-(supb, chunk) segmentation as g1 (descriptor-ring cap)
    t0 = 0
    for (sb, c, tiles) in sched["seg_tiles"]:
        seg = core["g2"][t0 * P:(t0 + tiles) * P]
        if tiles:
            g2_parts.append(_wrap_idx(seg))
        t0 += tiles
    g1w = np.concatenate(g1_parts, axis=1) if g1_parts else np.zeros((128, 0), np.int16)
    g2w = np.concatenate(g2_parts, axis=1) if g2_parts else np.zeros((128, 0), np.int16)
    dl = core["dl"].reshape(T_tot, P).T.astype(BF16).copy()
    return g1w, g2w, dl


# ---------------------------------------------------------------------------
def _build_nc(sched):
    caps = sched["caps"]
    supb_sizes = sched["supb_sizes"]
    T_tot = sched["T_tot"]
    AF = mybir.ActivationFunctionType
    AL = mybir.AluOpType
    f32, bf16, i16 = mybir.dt.float32, mybir.dt.bfloat16, mybir.dt.int16

    g1cols = sum(t * 8 for (_, _, t) in sched["seg_tiles"])
    g2cols = T_tot * 8

    nc = bacc.Bacc("TRN2")
    xT = nc.declare_dram_parameter("xT", [P, NP_], f32, isOutput=False)
    xsT = nc.declare_dram_parameter("xsT", [P, SHARD], f32, isOutput=False)
    Wp = nc.declare_dram_parameter("W", [P, P], f32, isOutput=False)
    Acat = nc.declare_dram_parameter("Acat", [P, 8], f32, isOutput=False)
    biasr = nc.declare_dram_parameter("biasr", [P, P], f32, isOutput=False)
    iotap = nc.declare_dram_parameter("iota", [P, P], bf16, isOutput=False)
    negrow = nc.declare_dram_parameter("negrow", [1, P], bf16, isOutput=False)
    g1i = nc.declare_dram_parameter("g1i", [P, max(g1cols, 8)], i16, isOutput=False)
    g2i = nc.declare_dram_parameter("g2i", [P, max(g2cols, 8)], i16, isOutput=False)
    dlp = nc.declare_dram_parameter("dloc", [P, max(T_tot, 1)], bf16, isOutput=False)
    outp = nc.declare_dram_parameter("out", [SHARD, P], f32, isOutput=True)

    T1 = nc.dram_tensor("t1", [NP_, 256], bf16)
    T2 = nc.dram_tensor("t2", [SHARD + 1, P], bf16)

    nc.gpsimd.load_library(mlp)

    with tile.TileContext(nc) as tc:
        with tc.tile_pool(name="const", bufs=1) as cpool:
            ident = cpool.tile([P, P], f32)
            make_identity(nc, ident[:])
            iot = cpool.tile([P, P], bf16)
            nc.sync.dma_start(out=iot[:], in_=iotap[:])
            bias_t = cpool.tile([P, P], f32)
            nc.sync.dma_start(out=bias_t[:], in_=biasr[:])
            wprime = cpool.tile([P, 136], f32)
            nc.sync.dma_start(out=wprime[:, 0:128], in_=Wp[:])
            acat_t = cpool.tile([P, 8], f32)
            nc.sync.dma_start(out=acat_t[:], in_=Acat[:])
            # ---------------- table build ----------------
            with tc.tile_pool(name="tb", bufs=3) as tb, \
                 tc.tile_pool(name="tbp", bufs=2, space="PSUM") as tbp:
                # W' cols 128:136 = W @ Acat  (contract over out-features)
                wtp = tbp.tile([P, P], f32, space="PSUM")
                nc.tensor.transpose(out=wtp[:], in_=wprime[:, 0:128], identity=ident[:])
                wT = tb.tile([P, P], f32)
                nc.vector.tensor_copy(out=wT[:], in_=wtp[:])
                wap = tbp.tile([P, 8], f32, space="PSUM")
                nc.tensor.matmul(out=wap[:], lhsT=wT[:], rhs=acat_t[:],
                                 start=True, stop=True)
                nc.vector.tensor_copy(out=wprime[:, 128:136], in_=wap[:])
                NBLK = 8
                for b in range(NP_ // P // NBLK):
                    xt = tb.tile([P, NBLK * P], f32, tag="xt")
                    nc.sync.dma_start(
                        out=xt[:], in_=xT[:, b * NBLK * P:(b + 1) * NBLK * P])
                    st = tb.tile([P, NBLK * 256], bf16, tag="st")
                    nc.gpsimd.memset(st[:], 0)
                    for t in range(NBLK):
                        ps = tbp.tile([P, 136], f32, space="PSUM", tag="ps")
                        nc.tensor.matmul(
                            out=ps[:], lhsT=xt[:, t * P:(t + 1) * P],
                            rhs=wprime[:], start=True, stop=True)
                        if t % 2 == 0:
                            nc.vector.tensor_copy(
                                out=st[:, t * 256:t * 256 + 136], in_=ps[:])
                        else:
                            nc.scalar.activation(
                                out=st[:, t * 256:t * 256 + 136], in_=ps[:],
                                func=AF.Copy)
                    nc.sync.dma_start(
                        out=T1[b * NBLK * P:(b + 1) * NBLK * P, :].rearrange(
                            "(t p) c -> p t c", p=P),
                        in_=st[:].rearrange("p (t c) -> p t c", t=NBLK))
                for j in range(SLOTS):
                    xt = tb.tile([P, P], f32, tag="xt")
                    nc.sync.dma_start(out=xt[:], in_=xsT[:, j * P:(j + 1) * P])
                    ps = tbp.tile([P, 136], f32, space="PSUM", tag="ps")
                    nc.tensor.matmul(out=ps[:], lhsT=xt[:], rhs=wprime[:],
                                     start=True, stop=True)
                    st2 = tb.tile([P, P], bf16, tag="st2")
                    nc.gpsimd.memset(st2[:, 4:128], 0)
                    nc.vector.tensor_copy(out=st2[:, 0:4], in_=ps[:, 132:136])
                    nc.sync.dma_start(out=T2[j * P:(j + 1) * P, :], in_=st2[:])
                ng = tb.tile([1, P], bf16, tag="ng")
                nc.sync.dma_start(out=ng[:], in_=negrow[:])
                nc.sync.dma_start(out=T2[PADROW:PADROW + 1, :], in_=ng[:])

            # ---------------- edge phase ----------------
            _PH = int(os.environ.get("GAT_PHASES", "3"))
            with tc.tile_pool(name="eg", bufs=2) as eg, \
                 tc.tile_pool(name="ew", bufs=4) as ew, \
                 tc.tile_pool(name="eo", bufs=2) as eo, \
                 tc.tile_pool(name="eps", bufs=1, space="PSUM") as epsum:
                g1_colcur = 0
                g2_colcur = 0
                tilecur = 0
                seg_iter = 0
                sb0 = 0
                for sb, nsl in enumerate(supb_sizes):
                    if _PH < 1:
                        break
                    sl = slice(sb0, sb0 + nsl)
                    ctiles = [int(caps[sl, c].sum()) for c in range(4)]
                    stiles = sum(ctiles)
                    if stiles == 0:
                        sb0 += nsl
                        seg_iter += 4
                        continue
                    # index + dstloc loads for this superblock
                    g2it = eg.tile([P, stiles * 8], i16, tag="g2it")
                    nc.scalar.dma_start(
                        out=g2it[:], in_=g2i[:, g2_colcur:g2_colcur + stiles * 8])
                    g2_colcur += stiles * 8
                    dlt = eg.tile([P, stiles], bf16, tag="dlt")
                    nc.scalar.dma_start(
                        out=dlt[:], in_=dlp[:, tilecur:tilecur + stiles])
                    # a_dst gather buffer; filled per chunk segment below
                    g2b = eg.tile([P, stiles * P], bf16, tag="g2b")
                    if _PH < 2:
                        nc.gpsimd.memset(g2b[:], 0)

                    psum_of_slot = {}
                    flags = {}
                    for j in range(nsl):
                        g = sb0 + j
                        live = [c for c in range(4) if caps[g, c] > 0]
                        if live:
                            flags[j] = (live[0], live[-1])
                            psum_of_slot[j] = epsum.tile(
                                [P, 132], f32, space="PSUM", name=f"pslot{j}", tag=f"ps{j}")

                    srun = 0   # tile index within the supb stream
                    for c in range(4):
                        Lc = ctiles[c]
                        if Lc == 0:
                            seg_iter += 1
                            continue
                        g1it = eg.tile([P, Lc * 8], i16, tag="g1it")
                        nc.scalar.dma_start(
                            out=g1it[:], in_=g1i[:, g1_colcur:g1_colcur + Lc * 8])
                        g1_colcur += Lc * 8
                        g1b = eg.tile([P, Lc * 256], bf16, tag="g1b", bufs=3)
                        if _PH < 2:
                            nc.gpsimd.memset(g1b[:], 0)
                        _PH < 2 or nc.gpsimd.dma_gather(
                            g1b[:].rearrange("p (t c) -> p t c", t=Lc),
                            T1[c * CHUNK:(c + 1) * CHUNK, :],
                            g1it[:], Lc * P, Lc * P, 256,
                            single_packet=False)
                        _PH < 2 or nc.gpsimd.dma_gather(
                            g2b[:].rearrange("p (t c) -> p t c", t=stiles)[
                                :, srun:srun + Lc, :],
                            T2[:], g2it[:, srun * 8:(srun + Lc) * 8],
                            Lc * P, Lc * P, P, single_packet=False)
                        g1v = g1b[:].rearrange("p (t c) -> p t c", t=Lc)
                        crun = 0   # tile within this chunk segment
                        for j in range(nsl):
                            g = sb0 + j
                            K = int(caps[g, c])
                            if K == 0:
                                continue
                            po = psum_of_slot[j]
                            # logits/weights for the whole run
                            z = ew.tile([P, K * 4], f32, tag="z")
                            nc.vector.tensor_tensor(
                                out=z[:].rearrange("p (t c) -> p t c", t=K),
                                in0=g1v[:, crun:crun + K, 128:132],
                                in1=g2b[:].rearrange("p (t c) -> p t c", t=stiles)[
                                    :, srun + crun:srun + crun + K, 0:4],
                                op=AL.add)
                            e1 = ew.tile([P, K * 4], f32, tag="e1")
                            nc.scalar.activation(out=e1[:], in_=z[:], func=AF.Exp)
                            e2 = ew.tile([P, K * 4], f32, tag="e2")
                            nc.scalar.activation(out=e2[:], in_=z[:], func=AF.Exp,
                                                 scale=0.2)
                            w = ew.tile([P, K * 4], f32, tag="w")
                            nc.vector.tensor_tensor(out=w[:], in0=e1[:], in1=e2[:],
                                                    op=AL.max)
                            wv = w[:].rearrange("p (t c) -> p t c", t=K)
                            for t in range(K):
                                gt = crun + t
                                sel = ew.tile([P, P], bf16, tag="sel")
                                nc.vector.tensor_tensor(
                                    out=sel[:],
                                    in0=dlt[:, srun + gt:srun + gt + 1].to_broadcast([P, P]),
                                    in1=iot[:], op=AL.is_equal)
                                mp = ew.tile([P, 132], bf16, tag="mp")
                                nc.vector.tensor_tensor(
                                    out=mp[:, 0:128].rearrange("p (h c) -> p h c", h=4),
                                    in0=g1v[:, gt, 0:128].rearrange("p (h c) -> p h c", h=4),
                                    in1=wv[:, t, :].unsqueeze(-1).to_broadcast([P, 4, 32]),
                                    op=AL.mult)
                                nc.scalar.activation(out=mp[:, 128:132], in_=wv[:, t, :],
                                                     func=AF.Copy)
                                fc = flags[j]
                                nc.tensor.matmul(
                                    out=po[:], lhsT=sel[:], rhs=mp[:],
                                    start=(c == fc[0] and t == 0),
                                    stop=(c == fc[1] and t == K - 1))
                            crun += K
                        srun += Lc
                        seg_iter += 1
                    # flush slots
                    for j in range(nsl):
                        if j not in psum_of_slot:
                            continue
                        g = sb0 + j
                        po = psum_of_slot[j]
                        rec = ew.tile([P, 4], f32, tag="rec")
                        nc.vector.reciprocal(out=rec[:], in_=po[:, 128:132])
                        ot = eo.tile([P, P], f32, tag="ot")
                        for hh in range(4):
                            nc.vector.tensor_scalar_mul(
                                ot[:, hh * 32:(hh + 1) * 32],
                                po[:, hh * 32:(hh + 1) * 32], rec[:, hh:hh + 1])
                        nc.vector.tensor_tensor(out=ot[:], in0=ot[:], in1=bias_t[:],
                                                op=AL.add)
                        nc.sync.dma_start(out=outp[g * P:(g + 1) * P, :], in_=ot[:])
                    tilecur += stiles
                    sb0 += nsl
    nc.compile()
    if not os.environ.get("BASS_NO_WAITSPLIT"):
        _split_multi_waits(nc)
    return nc


# ---------------------------------------------------------------------------
_BUILD_CACHE = {}


def _prep_and_build(x, edge_index, W, att_src, att_dst, bias):
    cores, sched = _host_prep(np.asarray(x), np.asarray(edge_index))
    nc = _build_nc(sched)

    x = np.asarray(x, np.float32)
    xpad = np.zeros((NP_, IN_DIM), np.float32)
    xpad[:N] = x
    xT = np.ascontiguousarray(xpad.T)

    Acat = np.zeros((P, 8), np.float32)
    a_s = np.asarray(att_src, np.float32)
    a_d = np.asarray(att_dst, np.float32)
    for h in range(HEADS):
        Acat[h * CDIM:(h + 1) * CDIM, h] = a_s[h]
        Acat[h * CDIM:(h + 1) * CDIM, 4 + h] = a_d[h]
    biasr = np.tile(np.asarray(bias, np.float32)[None, :], (P, 1))
    iota = np.tile(np.arange(P, dtype=BF16)[None, :], (P, 1))
    negrow = np.full((1, P), _NEG, BF16)
    Wf = np.ascontiguousarray(np.asarray(W, np.float32))

    in_maps = []
    for k in range(NCORES):
        g1w, g2w, dl = _pack_core_arrays(cores[k], sched)
        nodes = (sched["grp"][:, k][:, None] * P + np.arange(P)[None, :]).reshape(-1)
        xsT = np.ascontiguousarray(xpad[nodes].T)
        in_maps.append({
            "xT": xT, "xsT": xsT, "W": Wf, "Acat": Acat, "biasr": biasr,
            "iota": iota, "negrow": negrow,
            "g1i": np.ascontiguousarray(g1w), "g2i": np.ascontiguousarray(g2w),
            "dloc": np.ascontiguousarray(dl),
        })
    return nc, in_maps, sched


def _assemble(results, sched):
    full = np.zeros((NP_, P), np.float32)
    grp = sched["grp"]
    for k in range(NCORES):
        o = np.asarray(results[k]["out"])        # [SHARD, 128]
        wins = grp[:, k]                         # window id per slot
        full[(wins[:, None] * P + np.arange(P)[None, :]).reshape(-1)] = o
    return full[:N]


def kernel(**inputs):
    x = inputs["x"]
    edge_index = inputs["edge_index"]
    nc, in_maps, sched = _prep_and_build(
        x, edge_index, inputs["W"], inputs["att_src"], inputs["att_dst"],
        inputs["bias"])
    res = run_bass_kernel_spmd(nc, in_maps, core_ids=list(range(NCORES)))
    return _assemble(res.results, sched)
